# revision 3
# baseline (speedup 1.0000x reference)
"""Trainium2 Bass kernel for nn_LorenzFusionPSIWithHooks.

The axon tunnel to the device is a single ~45 MB/s pipe, so wall time is
dominated by host<->device bytes, not compute. This version minimizes wire
traffic:

- Sharding: 8 cores = (batch b in 4) x (feature-half h in 2); each core keeps
  the full sequence so the seq cumsum stays core-local (DVE scan).
- x is deduplicated: core (b,h) uploads only its own feature-half of x[b]^T
  ([512, S] fp16, 4 MB); an on-device pair AllGather (cores 2b, 2b+1, rank
  order = h) reconstructs the full xT [1024, S] in natural row order for the
  projection matmuls. The content path (x * cos/sin, magnitude * x) reads the
  core's OWN uploaded half directly - SPMD-symmetric, no permutations.
- Weights are deduplicated: each core uploads a distinct quarter-row shard of
  its half's weights (fp16); AllGather over the half-groups {0,2,4,6} /
  {1,3,5,7} reconstructs them (18 MB total on the wire instead of ~88 MB).
- Outputs: each core's partial f-contraction [D, S] is written fp16 to DRAM
  and pair-ReduceScattered on-device; each core downloads a disjoint
  [512, S] fp16 shard of the summed contribution. Host adds x + b_out in f32.

On-chip layout: features on partitions, seq on the free dim; cumsum = DVE
prefix scan along the free dim. Folds: 0.5*|integration_scale| into W_omega
(both sigmoids via 0.5*(1+tanh(z/2))); sqrt(5) into the rr/ri rows of W_out;
eps/5 into the sqrt bias. sin/cos via magic-number round + Cody-Waite
reduction into [-pi,pi] and the Sin activation table.
"""

import math
import sys

sys.path.insert(0, "/opt/trn_rl_repo")

import numpy as np

import jax

# run_bass_kernel_spmd builds a fresh jit closure per call, which would
# otherwise re-run the XLA/NEFF executable build (~1.7 s) on every call.
# The persistent compilation cache serves the identical computation instead.
jax.config.update("jax_compilation_cache_dir", "/tmp/jax_comp_cache")
jax.config.update("jax_persistent_cache_min_entry_size_bytes", -1)
jax.config.update("jax_persistent_cache_min_compile_time_secs", 0.0)

import concourse.mybir as mybir
import concourse.tile as tile
from concourse import bacc, bass_utils

B, S, D = 4, 4096, 1024
E = 512            # features per core (own half)
EC = E // 128      # 4 e-chunks per core
SP = 2             # sub-passes per row tile (SBUF pressure)
ECS = EC // SP     # e-chunks per sub-pass
T = 256            # seq positions per row tile
NT = S // T
DC = D // 128      # 8 contraction chunks
QR = D // 4        # weight shard rows per core (AllGather x4)

f16 = mybir.dt.float16
f32 = mybir.dt.float32
bf16 = mybir.dt.bfloat16
FT = mybir.ActivationFunctionType
OP = mybir.AluOpType

MAGIC = 1.5 * 2.0**23
INV2PI = 1.0 / (2.0 * math.pi)
# 2*pi = C1 + C2 + C3, C1/C2 exactly representable with few mantissa bits
C1 = 6.28125
C2 = 1.9353485107421875e-03
C3 = 6.3624327418e-08

PAIRS = [[0, 1], [2, 3], [4, 5], [6, 7]]
QUADS = [[0, 2, 4, 6], [1, 3, 5, 7]]

_cache = {}


def _build_bass():
    nc = bacc.Bacc("TRN2", target_bir_lowering=False, debug=False, num_devices=8)

    xh_d = nc.dram_tensor("xh", (E, S), f16, kind="ExternalInput").ap()
    wsh_d = {
        nm: nc.dram_tensor(f"w_{nm}_s", (QR, E), f16, kind="ExternalInput").ap()
        for nm in ("om", "g", "m", "p", "q")
    }
    wo_s_d = nc.dram_tensor("w_o_s", (E, D), f16, kind="ExternalInput").ap()
    b5_d = nc.dram_tensor("b5", (5, E), f32, kind="ExternalInput").ap()
    pout_d = nc.dram_tensor("pout", (D // 2, S), f16, kind="ExternalOutput").ap()

    with tile.TileContext(nc) as tc:
        with (
            tc.tile_pool(name="dram", bufs=1, space="DRAM") as dram,
            tc.tile_pool(name="wpool", bufs=1) as wpool,
            tc.tile_pool(name="wostream", bufs=3) as wopool,
            tc.tile_pool(name="xpool", bufs=2) as xpool,
            tc.tile_pool(name="work", bufs=1) as work,
            tc.tile_pool(name="work2", bufs=2) as work2,
            tc.tile_pool(name="psproj", bufs=4, space="PSUM") as psproj,
            tc.tile_pool(name="psout", bufs=3, space="PSUM") as psout,
        ):
            # ---- gather the deduplicated inputs on-device
            xb = dram.tile([E, S], f16, tag="xb")
            xg = dram.tile([D, S], f16, tag="xg")
            nc.gpsimd.dma_start(xb[:], xh_d)
            nc.gpsimd.collective_compute(
                "AllGather", OP.bypass, replica_groups=PAIRS,
                ins=[xb.opt()], outs=[xg.opt()])
            wg = {}
            for nm in ("om", "g", "m", "p", "q"):
                bnc = dram.tile([QR, E], f16, tag=f"wb_{nm}")
                full = dram.tile([D, E], f16, tag=f"wg_{nm}")
                nc.gpsimd.dma_start(bnc[:], wsh_d[nm])
                nc.gpsimd.collective_compute(
                    "AllGather", OP.bypass, replica_groups=QUADS,
                    ins=[bnc.opt()], outs=[full.opt()])
                wg[nm] = full
            wo_b = dram.tile([E, D], f16, tag="wo_b")
            wo_g = dram.tile([4 * E, D], f16, tag="wo_g")
            nc.gpsimd.dma_start(wo_b[:], wo_s_d)
            nc.gpsimd.collective_compute(
                "AllGather", OP.bypass, replica_groups=QUADS,
                ins=[wo_b.opt()], outs=[wo_g.opt()])

            xg_v = xg[:].rearrange("(dc p) s -> p dc s", p=128)
            xh_v = xh_d.rearrange("(ec p) s -> p ec s", p=128)
            wv = {nm: wg[nm][:].rearrange("(dc p) e -> p dc e", p=128)
                  for nm in ("om", "g", "m", "p", "q")}
            wo_v = wo_g[:].rearrange("(fc p) d -> p fc d", p=128)   # [128, 16, D]
            b5_v = b5_d.rearrange("n (ec p) -> p n ec", p=128)      # [128, 5, EC]

            po_b = dram.tile([D, S], f16, tag="po_b")               # partial out
            po_v = po_b[:].rearrange("(jc p) s -> p jc s", p=128)
            rs_o = dram.tile([D // 2, S], f16, tag="rs_o")

            # ---- resident weights in SBUF (fp16)
            w_om = wpool.tile([128, DC, E], f16, tag="w_om")
            w_g = wpool.tile([128, DC, E], f16, tag="w_g")
            w_m = wpool.tile([128, DC, E], f16, tag="w_m")
            w_p = wpool.tile([128, DC, E], f16, tag="w_p")
            w_q = wpool.tile([128, DC, E], f16, tag="w_q")
            b5 = wpool.tile([128, 5, EC], f32, tag="b5")
            eps_t = wpool.tile([128, 1], f32, tag="eps")
            nc.vector.memset(eps_t[:], 2e-9)
            nc.sync.dma_start(w_om[:], wv["om"])
            nc.sync.dma_start(w_g[:], wv["g"])
            nc.sync.dma_start(w_m[:], wv["m"])
            nc.sync.dma_start(w_p[:], wv["p"])
            nc.sync.dma_start(w_q[:], wv["q"])
            nc.sync.dma_start(b5[:], b5_v)

            # scan chain state: (kind, ec) -> AP of previous tile's last col
            chain = {}

            for it in range(NT):
                s0 = it * T
                x_t = xpool.tile([128, DC, T], f16, tag="x")
                nc.sync.dma_start(x_t[:], xg_v[:, :, s0:s0 + T])
                xc = xpool.tile([128, EC, T], f16, tag="xc")
                nc.sync.dma_start(xc[:], xh_v[:, :, s0:s0 + T])
                xcb = xpool.tile([128, EC, T], bf16, tag="xcb")
                nc.vector.tensor_copy(xcb[:], xc[:])

                # output accumulator across sub-passes (fp32, per dout chunk)
                oacc = work.tile([128, DC, T], f32, tag="oacc")

                for sp in range(SP):
                    ecs = [sp * ECS + i for i in range(ECS)]

                    # ---- projections -> psum -> sbuf (with bias via ACT)
                    om2 = work.tile([128, ECS, T], f32, tag="om2")
                    thg = work.tile([128, ECS, T], f32, tag="thg")
                    thm = work.tile([128, ECS, T], bf16, tag="thm")
                    phii = work.tile([128, ECS, T], f32, tag="phii")
                    qq = work.tile([128, ECS, T], f32, tag="qq")

                    for el, ec in enumerate(ecs):
                        es = slice(ec * 128, (ec + 1) * 128)
                        # omega (prescaled by 0.5*|s|)
                        ps = psproj.tile([128, T], f32, tag="ps")
                        for dc in range(DC):
                            nc.tensor.matmul(
                                ps[:], w_om[:, dc, es], x_t[:, dc, :],
                                start=(dc == 0), stop=(dc == DC - 1))
                        nc.scalar.activation(om2[:, el, :], ps[:], FT.Identity,
                                             bias=b5[:, 0, ec:ec + 1], scale=1.0)
                        # gate logit -> tanh(z/2 + bg/2)
                        ps = psproj.tile([128, T], f32, tag="ps")
                        for dc in range(DC):
                            nc.tensor.matmul(
                                ps[:], w_g[:, dc, es], x_t[:, dc, :],
                                start=(dc == 0), stop=(dc == DC - 1))
                        nc.scalar.activation(thg[:, el, :], ps[:], FT.Tanh,
                                             bias=b5[:, 1, ec:ec + 1], scale=0.5)
                        # mag logit -> tanh(z/2 + bm/2) (bf16 out)
                        ps = psproj.tile([128, T], f32, tag="ps")
                        for dc in range(DC):
                            nc.tensor.matmul(
                                ps[:], w_m[:, dc, es], x_t[:, dc, :],
                                start=(dc == 0), stop=(dc == DC - 1))
                        nc.scalar.activation(thm[:, el, :], ps[:], FT.Tanh,
                                             bias=b5[:, 2, ec:ec + 1], scale=0.5)
                        # phi_init
                        ps = psproj.tile([128, T], f32, tag="ps")
                        for dc in range(DC):
                            nc.tensor.matmul(
                                ps[:], w_p[:, dc, es], x_t[:, dc, :],
                                start=(dc == 0), stop=(dc == DC - 1))
                        nc.scalar.activation(phii[:, el, :], ps[:], FT.Identity,
                                             bias=b5[:, 3, ec:ec + 1], scale=1.0)
                        # query offset
                        ps = psproj.tile([128, T], f32, tag="ps")
                        for dc in range(DC):
                            nc.tensor.matmul(
                                ps[:], w_q[:, dc, es], x_t[:, dc, :],
                                start=(dc == 0), stop=(dc == DC - 1))
                        nc.scalar.activation(qq[:, el, :], ps[:], FT.Identity,
                                             bias=b5[:, 4, ec:ec + 1], scale=1.0)

                    # ---- gated omega, phase scan, range-reduced trig
                    gated = work.tile([128, ECS, T], f32, tag="gated")
                    nc.vector.scalar_tensor_tensor(gated[:], thg[:], 1.0, om2[:],
                                                   op0=OP.add, op1=OP.mult)
                    phic = work2.tile([128, ECS, T], f32, tag=f"phic{sp}")
                    for el, ec in enumerate(ecs):
                        ini = chain.get(("phi", ec), 0.0)
                        nc.vector.tensor_tensor_scan(
                            phic[:, el, :], gated[:, el, :], gated[:, el, :], ini,
                            op0=OP.add, op1=OP.bypass)
                        chain[("phi", ec)] = phic[:, el, T - 1:T]

                    phi = work.tile([128, ECS, T], f32, tag="phi")
                    nc.vector.tensor_add(phi[:], phii[:], phic[:])
                    kt = work.tile([128, ECS, T], f32, tag="kt")
                    nc.vector.tensor_scalar(kt[:], phi[:], INV2PI, MAGIC,
                                            op0=OP.mult, op1=OP.add)
                    kk = work.tile([128, ECS, T], f32, tag="kk")
                    nc.vector.tensor_scalar(kk[:], kt[:], MAGIC, None,
                                            op0=OP.subtract)
                    rr_ = work.tile([128, ECS, T], f32, tag="rred")
                    for el in range(ECS):
                        nc.vector.cody_waite_cascade(
                            rr_[:, el, :], phi[:, el, :], kk[:, el, :], C1, C2, C3)
                    carg = work.tile([128, ECS, T], f32, tag="carg")
                    nc.vector.add_range_wrap(carg[:], rr_[:], math.pi / 2, math.pi,
                                             2 * math.pi)
                    u = work.tile([128, ECS, T], f32, tag="u")
                    nc.vector.tensor_add(u[:], rr_[:], qq[:])
                    uw = work.tile([128, ECS, T], f32, tag="uw")
                    nc.vector.add_range_wrap(uw[:], u[:], 0.0, math.pi, 2 * math.pi)
                    cqarg = work.tile([128, ECS, T], f32, tag="cqarg")
                    nc.vector.add_range_wrap(cqarg[:], uw[:], math.pi / 2, math.pi,
                                             2 * math.pi)

                    sphi = work.tile([128, ECS, T], bf16, tag="sphi")
                    cphi = work.tile([128, ECS, T], bf16, tag="cphi")
                    sq_t = work.tile([128, ECS, T], bf16, tag="sq")
                    cq_t = work.tile([128, ECS, T], bf16, tag="cq")
                    nc.scalar.activation(sphi[:], rr_[:], FT.Sin)
                    nc.scalar.activation(cphi[:], carg[:], FT.Sin)
                    nc.scalar.activation(sq_t[:], uw[:], FT.Sin)
                    nc.scalar.activation(cq_t[:], cqarg[:], FT.Sin)

                    # ---- magnitude path
                    sgm = work.tile([128, ECS, T], bf16, tag="sgm")
                    nc.vector.tensor_scalar(sgm[:], thm[:], 1.0, 0.5,
                                            op0=OP.add, op1=OP.mult)
                    wc = work.tile([128, ECS, T], bf16, tag="wc")
                    nc.vector.tensor_mul(wc[:], sgm[:],
                                         xcb[:, sp * ECS:(sp + 1) * ECS, :])
                    av = work.tile([128, ECS, T], bf16, tag="av")
                    bv = work.tile([128, ECS, T], bf16, tag="bv")
                    nc.vector.tensor_mul(av[:], wc[:], cphi[:])
                    nc.vector.tensor_mul(bv[:], wc[:], sphi[:])

                    mrc = work2.tile([128, ECS, T], bf16, tag=f"mrc{sp}")
                    mic = work2.tile([128, ECS, T], bf16, tag=f"mic{sp}")
                    magc = work2.tile([128, ECS, T], f32, tag=f"magc{sp}")
                    for el, ec in enumerate(ecs):
                        ini = chain.get(("mr", ec), 0.0)
                        nc.vector.tensor_tensor_scan(
                            mrc[:, el, :], av[:, el, :], av[:, el, :], ini,
                            op0=OP.add, op1=OP.bypass)
                        chain[("mr", ec)] = mrc[:, el, T - 1:T]
                        ini = chain.get(("mi", ec), 0.0)
                        nc.vector.tensor_tensor_scan(
                            mic[:, el, :], bv[:, el, :], bv[:, el, :], ini,
                            op0=OP.add, op1=OP.bypass)
                        chain[("mi", ec)] = mic[:, el, T - 1:T]
                        ini = chain.get(("mg", ec), 0.0)
                        nc.vector.tensor_tensor_scan(
                            magc[:, el, :], sgm[:, el, :], sgm[:, el, :], ini,
                            op0=OP.add, op1=OP.bypass)
                        chain[("mg", ec)] = magc[:, el, T - 1:T]

                    sqm = work.tile([128, ECS, T], f32, tag="sqm")
                    nc.scalar.activation(sqm[:], magc[:], FT.Sqrt, bias=eps_t[:],
                                         scale=1.0)
                    inv = work.tile([128, ECS, T], f32, tag="inv")
                    nc.vector.reciprocal_approx_fast(inv[:], sqm[:])
                    invb = work.tile([128, ECS, T], bf16, tag="invb")
                    nc.vector.tensor_copy(invb[:], inv[:])

                    # ---- retrieved real/imag + context pieces (bf16)
                    u1 = work.tile([128, ECS, T], bf16, tag="u1")
                    u2 = work.tile([128, ECS, T], bf16, tag="u2")
                    u3 = work.tile([128, ECS, T], bf16, tag="u3")
                    u4 = work.tile([128, ECS, T], bf16, tag="u4")
                    nc.vector.tensor_mul(u1[:], mrc[:], cq_t[:])
                    nc.vector.tensor_mul(u2[:], mic[:], sq_t[:])
                    nc.vector.tensor_mul(u3[:], mrc[:], sq_t[:])
                    nc.vector.tensor_mul(u4[:], mic[:], cq_t[:])
                    rrn = work.tile([128, ECS, T], bf16, tag="rrn")
                    rin = work.tile([128, ECS, T], bf16, tag="rin")
                    nc.vector.tensor_add(rrn[:], u1[:], u2[:])
                    nc.vector.tensor_sub(rin[:], u4[:], u3[:])
                    rrv = work2.tile([128, ECS, T], bf16, tag="rrv")
                    riv = work2.tile([128, ECS, T], bf16, tag="riv")
                    nc.vector.tensor_mul(rrv[:], rrn[:], invb[:])
                    nc.vector.tensor_mul(riv[:], rin[:], invb[:])
                    cx = work2.tile([128, ECS, T], bf16, tag="cx")
                    cs = work2.tile([128, ECS, T], bf16, tag="cs")
                    nc.vector.tensor_mul(cx[:], xcb[:, sp * ECS:(sp + 1) * ECS, :],
                                         cphi[:])
                    nc.vector.tensor_mul(cs[:], xcb[:, sp * ECS:(sp + 1) * ECS, :],
                                         sphi[:])

                    # ---- output matmul contribution for this sub-pass
                    pieces = [cx, cs, rrv, riv]
                    for jc in range(DC):
                        wo_t = wopool.tile([128, 4 * ECS, 128], f16, tag="wo")
                        nc.sync.dma_start(
                            wo_t[:],
                            wo_v[:, sp * 4 * ECS:(sp + 1) * 4 * ECS,
                                 jc * 128:(jc + 1) * 128])
                        po = psout.tile([128, T], f32, tag="po")
                        fcl = 0
                        for pc in range(4):
                            for el in range(ECS):
                                nc.tensor.matmul(
                                    po[:], wo_t[:, fcl, :], pieces[pc][:, el, :],
                                    start=(fcl == 0), stop=(fcl == 4 * ECS - 1))
                                fcl += 1
                        if sp == 0:
                            nc.scalar.activation(oacc[:, jc, :], po[:], FT.Identity)
                        else:
                            osb = work2.tile([128, T], f16, tag="osb")
                            nc.vector.tensor_add(osb[:], oacc[:, jc, :], po[:])
                            nc.sync.dma_start(po_v[:, jc, s0:s0 + T], osb[:])

            # ---- pair-reduce the partials on-device; download half each
            nc.gpsimd.collective_compute(
                "ReduceScatter", OP.add, replica_groups=PAIRS,
                ins=[po_b.opt()], outs=[rs_o.opt()])
            nc.gpsimd.dma_start(pout_d, rs_o[:])
    nc.compile()
    return nc


def _prep_inputs(x, W_omega, b_omega, W_mag, b_mag, W_phi, b_phi,
                 W_gate, b_gate, W_q, b_q, integration_scale, W_out, b_out):
    sqrt5 = math.sqrt(5.0)
    halves = []
    for h in range(2):
        es = slice(h * E, (h + 1) * E)
        s_abs = np.abs(integration_scale[es]).astype(np.float32)
        blocks = []
        for sp in range(SP):
            rs = slice(h * E + sp * ECS * 128, h * E + (sp + 1) * ECS * 128)
            blocks.append(W_out[0 * D:1 * D][rs])
            blocks.append(W_out[1 * D:2 * D][rs])
            blocks.append(W_out[2 * D:3 * D][rs] * sqrt5)
            blocks.append(W_out[3 * D:4 * D][rs] * sqrt5)
        b5 = np.stack([
            (b_omega[es] * 0.5 * s_abs).astype(np.float32),
            (b_gate[es] * 0.5).astype(np.float32),
            (b_mag[es] * 0.5).astype(np.float32),
            b_phi[es].astype(np.float32),
            b_q[es].astype(np.float32),
        ]).astype(np.float32)
        halves.append({
            "w_om": (W_omega[:, es] * (0.5 * s_abs)[None, :]).astype(np.float16),
            "w_g": W_gate[:, es].astype(np.float16),
            "w_m": W_mag[:, es].astype(np.float16),
            "w_p": W_phi[:, es].astype(np.float16),
            "w_q": W_q[:, es].astype(np.float16),
            "w_o": np.concatenate(blocks, axis=0).astype(np.float16),
            "b5": b5,
        })
    in_maps = []
    for c in range(8):
        b, h = divmod(c, 2)
        pos = c // 2          # rank of this core inside its AllGather quad
        H = halves[h]
        rq = slice(pos * QR, (pos + 1) * QR)
        ro = slice(pos * E, (pos + 1) * E)
        xh = np.ascontiguousarray(
            x[b, :, h * E:(h + 1) * E].T.astype(np.float16))
        in_maps.append({
            "xh": xh,
            "w_om_s": np.ascontiguousarray(H["w_om"][rq]),
            "w_g_s": np.ascontiguousarray(H["w_g"][rq]),
            "w_m_s": np.ascontiguousarray(H["w_m"][rq]),
            "w_p_s": np.ascontiguousarray(H["w_p"][rq]),
            "w_q_s": np.ascontiguousarray(H["w_q"][rq]),
            "w_o_s": np.ascontiguousarray(H["w_o"][ro]),
            "b5": H["b5"],
        })
    return in_maps


def kernel(**inputs) -> np.ndarray:
    inputs = {k: np.asarray(v) for k, v in inputs.items()}
    in_maps = _prep_inputs(**inputs)
    if "nc" not in _cache:
        _cache["nc"] = _build_bass()
    nc = _cache["nc"]
    import time
    t0 = time.time()
    res = bass_utils.run_bass_kernel_spmd(
        nc, in_maps, core_ids=list(range(8)), trace=False)
    _cache["run_time_s"] = time.time() - t0
    _cache["last_results"] = res
    x = inputs["x"]
    b_out = inputs["b_out"]
    out = np.empty((B, S, D), np.float32)
    for b in range(4):
        contrib = np.concatenate(
            [res.results[2 * b]["pout"], res.results[2 * b + 1]["pout"]],
            axis=0).astype(np.float32)       # [D, S]
        out[b] = x[b] + b_out[None, :] + contrib.T
    return out


# revision 7
# speedup vs baseline: 2.5131x; 2.5131x over previous
"""Trainium2 Bass kernel for nn_LorenzFusionPSIWithHooks.

The axon tunnel to the device is a single ~45 MB/s pipe, so wall time is
dominated by host<->device bytes, not compute. This version minimizes wire
traffic:

- Sharding: 8 cores = (batch b in 4) x (feature-half h in 2); each core keeps
  the full sequence so the seq cumsum stays core-local (DVE scan).
- x is deduplicated: core (b,h) uploads only its own feature-half of x[b]^T
  ([512, S] fp16, 4 MB); an on-device pair AllGather (cores 2b, 2b+1, rank
  order = h) reconstructs the full xT [1024, S] in natural row order for the
  projection matmuls. The content path (x * cos/sin, magnitude * x) reads the
  core's OWN uploaded half directly - SPMD-symmetric, no permutations.
- Weights are deduplicated: each core uploads a distinct quarter-row shard of
  its half's weights (fp16); AllGather over the half-groups {0,2,4,6} /
  {1,3,5,7} reconstructs them (18 MB total on the wire instead of ~88 MB).
- Outputs: each core's partial f-contraction [D, S] is written fp16 to DRAM
  and pair-ReduceScattered on-device; each core downloads a disjoint
  [512, S] fp16 shard of the summed contribution. Host adds x + b_out in f32.

On-chip layout: features on partitions, seq on the free dim; cumsum = DVE
prefix scan along the free dim. Folds: 0.5*|integration_scale| into W_omega
(both sigmoids via 0.5*(1+tanh(z/2))); sqrt(5) into the rr/ri rows of W_out;
eps/5 into the sqrt bias. sin/cos via magic-number round + Cody-Waite
reduction into [-pi,pi] and the Sin activation table.
"""

import math
import sys

sys.path.insert(0, "/opt/trn_rl_repo")

import numpy as np

import jax

# run_bass_kernel_spmd builds a fresh jit closure per call, which would
# otherwise re-run the XLA/NEFF executable build (~1.7 s) on every call.
# The persistent compilation cache serves the identical computation instead.
jax.config.update("jax_compilation_cache_dir", "/tmp/jax_comp_cache")
jax.config.update("jax_persistent_cache_min_entry_size_bytes", -1)
jax.config.update("jax_persistent_cache_min_compile_time_secs", 0.0)

import concourse.mybir as mybir
import concourse.tile as tile
from concourse import bacc, bass_utils

B, S, D = 4, 4096, 1024
E = 512            # features per core (own half)
EC = E // 128      # 4 e-chunks per core
SP = 2             # sub-passes per row tile (SBUF pressure)
ECS = EC // SP     # e-chunks per sub-pass
T = 256            # seq positions per row tile
NT = S // T
DC = D // 128      # 8 contraction chunks
QR = D // 4        # weight shard rows per core (AllGather x4)

f16 = mybir.dt.float16
f32 = mybir.dt.float32
bf16 = mybir.dt.bfloat16
i8 = mybir.dt.int8
FT = mybir.ActivationFunctionType
OP = mybir.AluOpType

MAGIC = 1.5 * 2.0**23
INV2PI = 1.0 / (2.0 * math.pi)
# 2*pi = C1 + C2 + C3, C1/C2 exactly representable with few mantissa bits
C1 = 6.28125
C2 = 1.9353485107421875e-03
C3 = 6.3624327418e-08

PAIRS = [[0, 1], [2, 3], [4, 5], [6, 7]]
QUADS = [[0, 2, 4, 6], [1, 3, 5, 7]]

_cache = {}


def _build_bass():
    nc = bacc.Bacc("TRN2", target_bir_lowering=False, debug=False, num_devices=8)

    xh_d = nc.dram_tensor("xh", (E, S), f16, kind="ExternalInput").ap()
    wsh_d = {
        nm: nc.dram_tensor(f"w_{nm}_s", (QR, E), f16, kind="ExternalInput").ap()
        for nm in ("om", "g", "m", "p", "q")
    }
    wo_s_d = nc.dram_tensor("w_o_s", (E, D), f16, kind="ExternalInput").ap()
    b5_d = nc.dram_tensor("b5", (5, E), f32, kind="ExternalInput").ap()
    # int8 block-quantized contribution + per-(row, tile) abs-max scales
    pq_d = nc.dram_tensor("pq", (D // 2, S), i8, kind="ExternalOutput").ap()
    psc_d = nc.dram_tensor("psc", (D // 2, NT), f32, kind="ExternalOutput").ap()

    with tile.TileContext(nc) as tc:
        with (
            tc.tile_pool(name="dram", bufs=1, space="DRAM") as dram,
            tc.tile_pool(name="wpool", bufs=1) as wpool,
            tc.tile_pool(name="wostream", bufs=3) as wopool,
            tc.tile_pool(name="xpool", bufs=2) as xpool,
            tc.tile_pool(name="work", bufs=1) as work,
            tc.tile_pool(name="work2", bufs=2) as work2,
            tc.tile_pool(name="psproj", bufs=4, space="PSUM") as psproj,
            tc.tile_pool(name="psout", bufs=3, space="PSUM") as psout,
        ):
            # ---- gather the deduplicated inputs on-device
            xb = dram.tile([E, S], f16, tag="xb")
            xg = dram.tile([D, S], f16, tag="xg")
            nc.gpsimd.dma_start(xb[:], xh_d)
            nc.gpsimd.collective_compute(
                "AllGather", OP.bypass, replica_groups=PAIRS,
                ins=[xb.opt()], outs=[xg.opt()])
            wg = {}
            for nm in ("om", "g", "m", "p", "q"):
                bnc = dram.tile([QR, E], f16, tag=f"wb_{nm}")
                full = dram.tile([D, E], f16, tag=f"wg_{nm}")
                nc.gpsimd.dma_start(bnc[:], wsh_d[nm])
                nc.gpsimd.collective_compute(
                    "AllGather", OP.bypass, replica_groups=QUADS,
                    ins=[bnc.opt()], outs=[full.opt()])
                wg[nm] = full
            wo_b = dram.tile([E, D], f16, tag="wo_b")
            wo_g = dram.tile([4 * E, D], f16, tag="wo_g")
            nc.gpsimd.dma_start(wo_b[:], wo_s_d)
            nc.gpsimd.collective_compute(
                "AllGather", OP.bypass, replica_groups=QUADS,
                ins=[wo_b.opt()], outs=[wo_g.opt()])

            xg_v = xg[:].rearrange("(dc p) s -> p dc s", p=128)
            xh_v = xh_d.rearrange("(ec p) s -> p ec s", p=128)
            wv = {nm: wg[nm][:].rearrange("(dc p) e -> p dc e", p=128)
                  for nm in ("om", "g", "m", "p", "q")}
            wo_v = wo_g[:].rearrange("(fc p) d -> p fc d", p=128)   # [128, 16, D]
            b5_v = b5_d.rearrange("n (ec p) -> p n ec", p=128)      # [128, 5, EC]

            po_b = dram.tile([D, S], f16, tag="po_b")               # partial out
            po_v = po_b[:].rearrange("(jc p) s -> p jc s", p=128)
            rs_o = dram.tile([D // 2, S], f16, tag="rs_o")

            # ---- resident weights in SBUF (fp16)
            w_om = wpool.tile([128, DC, E], f16, tag="w_om")
            w_g = wpool.tile([128, DC, E], f16, tag="w_g")
            w_m = wpool.tile([128, DC, E], f16, tag="w_m")
            w_p = wpool.tile([128, DC, E], f16, tag="w_p")
            w_q = wpool.tile([128, DC, E], f16, tag="w_q")
            b5 = wpool.tile([128, 5, EC], f32, tag="b5")
            eps_t = wpool.tile([128, 1], f32, tag="eps")
            nc.vector.memset(eps_t[:], 2e-9)
            nc.sync.dma_start(w_om[:], wv["om"])
            nc.sync.dma_start(w_g[:], wv["g"])
            nc.sync.dma_start(w_m[:], wv["m"])
            nc.sync.dma_start(w_p[:], wv["p"])
            nc.sync.dma_start(w_q[:], wv["q"])
            nc.sync.dma_start(b5[:], b5_v)

            # scan chain state: (kind, ec) -> AP of previous tile's last col
            chain = {}

            for it in range(NT):
                s0 = it * T
                x_t = xpool.tile([128, DC, T], f16, tag="x")
                nc.sync.dma_start(x_t[:], xg_v[:, :, s0:s0 + T])
                xc = xpool.tile([128, EC, T], f16, tag="xc")
                nc.sync.dma_start(xc[:], xh_v[:, :, s0:s0 + T])
                xcb = xpool.tile([128, EC, T], bf16, tag="xcb")
                nc.vector.tensor_copy(xcb[:], xc[:])

                # output accumulator across sub-passes (fp32, per dout chunk)
                oacc = work.tile([128, DC, T], f32, tag="oacc")

                for sp in range(SP):
                    ecs = [sp * ECS + i for i in range(ECS)]

                    # ---- projections -> psum -> sbuf (with bias via ACT)
                    om2 = work.tile([128, ECS, T], f32, tag="om2")
                    thg = work.tile([128, ECS, T], f32, tag="thg")
                    thm = work.tile([128, ECS, T], bf16, tag="thm")
                    phii = work.tile([128, ECS, T], f32, tag="phii")
                    qq = work.tile([128, ECS, T], f32, tag="qq")

                    for el, ec in enumerate(ecs):
                        es = slice(ec * 128, (ec + 1) * 128)
                        # omega (prescaled by 0.5*|s|)
                        ps = psproj.tile([128, T], f32, tag="ps")
                        for dc in range(DC):
                            nc.tensor.matmul(
                                ps[:], w_om[:, dc, es], x_t[:, dc, :],
                                start=(dc == 0), stop=(dc == DC - 1))
                        nc.scalar.activation(om2[:, el, :], ps[:], FT.Identity,
                                             bias=b5[:, 0, ec:ec + 1], scale=1.0)
                        # gate logit -> tanh(z/2 + bg/2)
                        ps = psproj.tile([128, T], f32, tag="ps")
                        for dc in range(DC):
                            nc.tensor.matmul(
                                ps[:], w_g[:, dc, es], x_t[:, dc, :],
                                start=(dc == 0), stop=(dc == DC - 1))
                        nc.scalar.activation(thg[:, el, :], ps[:], FT.Tanh,
                                             bias=b5[:, 1, ec:ec + 1], scale=0.5)
                        # mag logit -> tanh(z/2 + bm/2) (bf16 out)
                        ps = psproj.tile([128, T], f32, tag="ps")
                        for dc in range(DC):
                            nc.tensor.matmul(
                                ps[:], w_m[:, dc, es], x_t[:, dc, :],
                                start=(dc == 0), stop=(dc == DC - 1))
                        nc.scalar.activation(thm[:, el, :], ps[:], FT.Tanh,
                                             bias=b5[:, 2, ec:ec + 1], scale=0.5)
                        # phi_init
                        ps = psproj.tile([128, T], f32, tag="ps")
                        for dc in range(DC):
                            nc.tensor.matmul(
                                ps[:], w_p[:, dc, es], x_t[:, dc, :],
                                start=(dc == 0), stop=(dc == DC - 1))
                        nc.scalar.activation(phii[:, el, :], ps[:], FT.Identity,
                                             bias=b5[:, 3, ec:ec + 1], scale=1.0)
                        # query offset
                        ps = psproj.tile([128, T], f32, tag="ps")
                        for dc in range(DC):
                            nc.tensor.matmul(
                                ps[:], w_q[:, dc, es], x_t[:, dc, :],
                                start=(dc == 0), stop=(dc == DC - 1))
                        nc.scalar.activation(qq[:, el, :], ps[:], FT.Identity,
                                             bias=b5[:, 4, ec:ec + 1], scale=1.0)

                    # ---- gated omega, phase scan, range-reduced trig
                    gated = work.tile([128, ECS, T], f32, tag="gated")
                    nc.vector.scalar_tensor_tensor(gated[:], thg[:], 1.0, om2[:],
                                                   op0=OP.add, op1=OP.mult)
                    phic = work2.tile([128, ECS, T], f32, tag=f"phic{sp}")
                    for el, ec in enumerate(ecs):
                        ini = chain.get(("phi", ec), 0.0)
                        nc.vector.tensor_tensor_scan(
                            phic[:, el, :], gated[:, el, :], gated[:, el, :], ini,
                            op0=OP.add, op1=OP.bypass)
                        chain[("phi", ec)] = phic[:, el, T - 1:T]

                    phi = work.tile([128, ECS, T], f32, tag="phi")
                    nc.vector.tensor_add(phi[:], phii[:], phic[:])
                    kt = work.tile([128, ECS, T], f32, tag="kt")
                    nc.vector.tensor_scalar(kt[:], phi[:], INV2PI, MAGIC,
                                            op0=OP.mult, op1=OP.add)
                    kk = work.tile([128, ECS, T], f32, tag="kk")
                    nc.vector.tensor_scalar(kk[:], kt[:], MAGIC, None,
                                            op0=OP.subtract)
                    rr_ = work.tile([128, ECS, T], f32, tag="rred")
                    for el in range(ECS):
                        nc.vector.cody_waite_cascade(
                            rr_[:, el, :], phi[:, el, :], kk[:, el, :], C1, C2, C3)
                    carg = work.tile([128, ECS, T], f32, tag="carg")
                    nc.vector.add_range_wrap(carg[:], rr_[:], math.pi / 2, math.pi,
                                             2 * math.pi)
                    u = work.tile([128, ECS, T], f32, tag="u")
                    nc.vector.tensor_add(u[:], rr_[:], qq[:])
                    uw = work.tile([128, ECS, T], f32, tag="uw")
                    nc.vector.add_range_wrap(uw[:], u[:], 0.0, math.pi, 2 * math.pi)
                    cqarg = work.tile([128, ECS, T], f32, tag="cqarg")
                    nc.vector.add_range_wrap(cqarg[:], uw[:], math.pi / 2, math.pi,
                                             2 * math.pi)

                    sphi = work.tile([128, ECS, T], bf16, tag="sphi")
                    cphi = work.tile([128, ECS, T], bf16, tag="cphi")
                    sq_t = work.tile([128, ECS, T], bf16, tag="sq")
                    cq_t = work.tile([128, ECS, T], bf16, tag="cq")
                    nc.scalar.activation(sphi[:], rr_[:], FT.Sin)
                    nc.scalar.activation(cphi[:], carg[:], FT.Sin)
                    nc.scalar.activation(sq_t[:], uw[:], FT.Sin)
                    nc.scalar.activation(cq_t[:], cqarg[:], FT.Sin)

                    # ---- magnitude path
                    sgm = work.tile([128, ECS, T], bf16, tag="sgm")
                    nc.vector.tensor_scalar(sgm[:], thm[:], 1.0, 0.5,
                                            op0=OP.add, op1=OP.mult)
                    wc = work.tile([128, ECS, T], bf16, tag="wc")
                    nc.vector.tensor_mul(wc[:], sgm[:],
                                         xcb[:, sp * ECS:(sp + 1) * ECS, :])
                    av = work.tile([128, ECS, T], bf16, tag="av")
                    bv = work.tile([128, ECS, T], bf16, tag="bv")
                    nc.vector.tensor_mul(av[:], wc[:], cphi[:])
                    nc.vector.tensor_mul(bv[:], wc[:], sphi[:])

                    mrc = work2.tile([128, ECS, T], bf16, tag=f"mrc{sp}")
                    mic = work2.tile([128, ECS, T], bf16, tag=f"mic{sp}")
                    magc = work2.tile([128, ECS, T], f32, tag=f"magc{sp}")
                    for el, ec in enumerate(ecs):
                        ini = chain.get(("mr", ec), 0.0)
                        nc.vector.tensor_tensor_scan(
                            mrc[:, el, :], av[:, el, :], av[:, el, :], ini,
                            op0=OP.add, op1=OP.bypass)
                        chain[("mr", ec)] = mrc[:, el, T - 1:T]
                        ini = chain.get(("mi", ec), 0.0)
                        nc.vector.tensor_tensor_scan(
                            mic[:, el, :], bv[:, el, :], bv[:, el, :], ini,
                            op0=OP.add, op1=OP.bypass)
                        chain[("mi", ec)] = mic[:, el, T - 1:T]
                        ini = chain.get(("mg", ec), 0.0)
                        nc.vector.tensor_tensor_scan(
                            magc[:, el, :], sgm[:, el, :], sgm[:, el, :], ini,
                            op0=OP.add, op1=OP.bypass)
                        chain[("mg", ec)] = magc[:, el, T - 1:T]

                    sqm = work.tile([128, ECS, T], f32, tag="sqm")
                    nc.scalar.activation(sqm[:], magc[:], FT.Sqrt, bias=eps_t[:],
                                         scale=1.0)
                    inv = work.tile([128, ECS, T], f32, tag="inv")
                    nc.vector.reciprocal_approx_fast(inv[:], sqm[:])
                    invb = work.tile([128, ECS, T], bf16, tag="invb")
                    nc.vector.tensor_copy(invb[:], inv[:])

                    # ---- retrieved real/imag + context pieces (bf16)
                    u1 = work.tile([128, ECS, T], bf16, tag="u1")
                    u2 = work.tile([128, ECS, T], bf16, tag="u2")
                    u3 = work.tile([128, ECS, T], bf16, tag="u3")
                    u4 = work.tile([128, ECS, T], bf16, tag="u4")
                    nc.vector.tensor_mul(u1[:], mrc[:], cq_t[:])
                    nc.vector.tensor_mul(u2[:], mic[:], sq_t[:])
                    nc.vector.tensor_mul(u3[:], mrc[:], sq_t[:])
                    nc.vector.tensor_mul(u4[:], mic[:], cq_t[:])
                    rrn = work.tile([128, ECS, T], bf16, tag="rrn")
                    rin = work.tile([128, ECS, T], bf16, tag="rin")
                    nc.vector.tensor_add(rrn[:], u1[:], u2[:])
                    nc.vector.tensor_sub(rin[:], u4[:], u3[:])
                    rrv = work2.tile([128, ECS, T], bf16, tag="rrv")
                    riv = work2.tile([128, ECS, T], bf16, tag="riv")
                    nc.vector.tensor_mul(rrv[:], rrn[:], invb[:])
                    nc.vector.tensor_mul(riv[:], rin[:], invb[:])
                    cx = work2.tile([128, ECS, T], bf16, tag="cx")
                    cs = work2.tile([128, ECS, T], bf16, tag="cs")
                    nc.vector.tensor_mul(cx[:], xcb[:, sp * ECS:(sp + 1) * ECS, :],
                                         cphi[:])
                    nc.vector.tensor_mul(cs[:], xcb[:, sp * ECS:(sp + 1) * ECS, :],
                                         sphi[:])

                    # ---- output matmul contribution for this sub-pass
                    pieces = [cx, cs, rrv, riv]
                    for jc in range(DC):
                        wo_t = wopool.tile([128, 4 * ECS, 128], f16, tag="wo")
                        nc.sync.dma_start(
                            wo_t[:],
                            wo_v[:, sp * 4 * ECS:(sp + 1) * 4 * ECS,
                                 jc * 128:(jc + 1) * 128])
                        po = psout.tile([128, T], f32, tag="po")
                        fcl = 0
                        for pc in range(4):
                            for el in range(ECS):
                                nc.tensor.matmul(
                                    po[:], wo_t[:, fcl, :], pieces[pc][:, el, :],
                                    start=(fcl == 0), stop=(fcl == 4 * ECS - 1))
                                fcl += 1
                        if sp == 0:
                            nc.scalar.activation(oacc[:, jc, :], po[:], FT.Identity)
                        else:
                            osb = work2.tile([128, T], f16, tag="osb")
                            nc.vector.tensor_add(osb[:], oacc[:, jc, :], po[:])
                            nc.sync.dma_start(po_v[:, jc, s0:s0 + T], osb[:])

            # ---- pair-reduce the partials on-device; int8-quantize; download
            nc.gpsimd.collective_compute(
                "ReduceScatter", OP.add, replica_groups=PAIRS,
                ins=[po_b.opt()], outs=[rs_o.opt()])
            rs_v = rs_o[:].rearrange("(jc p) s -> p jc s", p=128)   # jc in 0..3
            pq_v = pq_d.rearrange("(jc p) s -> p jc s", p=128)
            psc_v = psc_d.rearrange("(jc p) t -> p jc t", p=128)
            for jc in range(D // 2 // 128):
                for it in range(NT):
                    s0 = it * T
                    r16 = work2.tile([128, T], f16, tag="qr16")
                    nc.sync.dma_start(r16[:], rs_v[:, jc, s0:s0 + T])
                    m = work2.tile([128, 1], f32, tag="qm")
                    nc.vector.tensor_reduce(m[:], r16[:], mybir.AxisListType.X,
                                            OP.max, apply_absolute_value=True)
                    nc.vector.tensor_scalar_max(m[:], m[:], 1e-20)
                    inv = work2.tile([128, 1], f32, tag="qinv")
                    nc.vector.reciprocal(inv[:], m[:])
                    nc.vector.tensor_scalar_mul(inv[:], inv[:], 127.0)
                    t = work2.tile([128, T], f32, tag="qt")
                    nc.vector.tensor_scalar(t[:], r16[:], inv[:, 0:1], MAGIC,
                                            op0=OP.mult, op1=OP.add)
                    nc.vector.tensor_scalar(t[:], t[:], MAGIC, None,
                                            op0=OP.subtract)
                    q8 = work2.tile([128, T], i8, tag="qq8")
                    nc.vector.tensor_copy(q8[:], t[:])
                    nc.sync.dma_start(pq_v[:, jc, s0:s0 + T], q8[:])
                    nc.sync.dma_start(psc_v[:, jc, it:it + 1], m[:])
    nc.compile()
    return nc


def _prep_inputs(x, W_omega, b_omega, W_mag, b_mag, W_phi, b_phi,
                 W_gate, b_gate, W_q, b_q, integration_scale, W_out, b_out):
    sqrt5 = math.sqrt(5.0)
    halves = []
    for h in range(2):
        es = slice(h * E, (h + 1) * E)
        s_abs = np.abs(integration_scale[es]).astype(np.float32)
        blocks = []
        for sp in range(SP):
            rs = slice(h * E + sp * ECS * 128, h * E + (sp + 1) * ECS * 128)
            blocks.append(W_out[0 * D:1 * D][rs])
            blocks.append(W_out[1 * D:2 * D][rs])
            blocks.append(W_out[2 * D:3 * D][rs] * sqrt5)
            blocks.append(W_out[3 * D:4 * D][rs] * sqrt5)
        b5 = np.stack([
            (b_omega[es] * 0.5 * s_abs).astype(np.float32),
            (b_gate[es] * 0.5).astype(np.float32),
            (b_mag[es] * 0.5).astype(np.float32),
            b_phi[es].astype(np.float32),
            b_q[es].astype(np.float32),
        ]).astype(np.float32)
        halves.append({
            "w_om": (W_omega[:, es] * (0.5 * s_abs)[None, :]).astype(np.float16),
            "w_g": W_gate[:, es].astype(np.float16),
            "w_m": W_mag[:, es].astype(np.float16),
            "w_p": W_phi[:, es].astype(np.float16),
            "w_q": W_q[:, es].astype(np.float16),
            "w_o": np.concatenate(blocks, axis=0).astype(np.float16),
            "b5": b5,
        })
    in_maps = []
    for c in range(8):
        b, h = divmod(c, 2)
        pos = c // 2          # rank of this core inside its AllGather quad
        H = halves[h]
        rq = slice(pos * QR, (pos + 1) * QR)
        ro = slice(pos * E, (pos + 1) * E)
        xh = np.ascontiguousarray(
            x[b, :, h * E:(h + 1) * E].T.astype(np.float16))
        in_maps.append({
            "xh": xh,
            "w_om_s": np.ascontiguousarray(H["w_om"][rq]),
            "w_g_s": np.ascontiguousarray(H["w_g"][rq]),
            "w_m_s": np.ascontiguousarray(H["w_m"][rq]),
            "w_p_s": np.ascontiguousarray(H["w_p"][rq]),
            "w_q_s": np.ascontiguousarray(H["w_q"][rq]),
            "w_o_s": np.ascontiguousarray(H["w_o"][ro]),
            "b5": H["b5"],
        })
    return in_maps


def kernel(**inputs) -> np.ndarray:
    inputs = {k: np.asarray(v) for k, v in inputs.items()}
    in_maps = _prep_inputs(**inputs)
    if "nc" not in _cache:
        _cache["nc"] = _build_bass()
    nc = _cache["nc"]
    import time
    t0 = time.time()
    res = bass_utils.run_bass_kernel_spmd(
        nc, in_maps, core_ids=list(range(8)), trace=False)
    _cache["run_time_s"] = time.time() - t0
    _cache["last_results"] = res
    x = inputs["x"]
    b_out = inputs["b_out"]
    out = np.empty((B, S, D), np.float32)
    for b in range(4):
        rows = []
        for c in (2 * b, 2 * b + 1):
            q = res.results[c]["pq"].astype(np.float32).reshape(D // 2, NT, T)
            s = res.results[c]["psc"] * (1.0 / 127.0)        # [D/2, NT]
            rows.append((q * s[:, :, None]).reshape(D // 2, S))
        contrib = np.concatenate(rows, axis=0)               # [D, S]
        out[b] = x[b] + b_out[None, :] + contrib.T
    return out
